# revision 3
# baseline (speedup 1.0000x reference)
"""Trainium2 Bass kernel for nn_AUAttnProcessor (AU-token attention processor).

Sharding: 8 cores = (batch b, head-group hg). Core c handles batch c//2 and
heads [4*(c%2), 4*(c%2)+4) (Ch=320 of C=640 channels).  Wq/Wk/Wv/Wak/Wav are
column-sharded, Wo row-sharded; each core emits a partial [S, C] output and the
host reduces the two partials per batch and adds bias + residual.

Per-core math (transposed orientation, flash-attention style):
  qT/kT = (Wslice.T @ hsT)           [80, S] per head   (contraction C on partitions)
  v     = hs @ Wv_slice              [S, 320]  stored bf16 with a ones column
  scoresT[k-chunk] = kT_chunk.T-matmul(qT)  -> PSUM [128, QB]
  expT = Exp(scoresT)  (no max subtraction; inputs are O(1))  -> SBUF bf16
  outT_aug += v_aug_chunk.T @ expT   [97, QB] PSUM; row 96 = softmax denominator
  merged = outT/sums + au_outT/au_sums   (gamma folded into Wav on host)
  partial = merged.T @ Wo_rows       (bf16)
"""

import os
import sys

import numpy as np

for _p in ("/opt/trn_rl_repo",):
    if os.path.isdir(_p) and _p not in sys.path:
        sys.path.insert(0, _p)

import concourse.bass as bass
import concourse.tile as tile
from concourse import bacc, mybir
from concourse.bass_utils import run_bass_kernel_spmd

# Problem dims
B, S, C, H, D = 4, 2048, 640, 8, 80
NH = 4            # heads per core
CH = NH * D       # 320 channels per core
KC = C // 128     # 5 contraction chunks
SC = S // 128     # 16 sequence chunks
NAU = 13          # 12 AU tokens + 1 null token
NAUP = 14         # padded to even for fp32r matmul ISA rules (pad row is zero)
QB = 1024         # q-block width for main attention
NQB = S // QB
SCALE = float(D) ** -0.5

F32 = mybir.dt.float32
F32R = mybir.dt.float32r
BF16 = mybir.dt.bfloat16
EXP = mybir.ActivationFunctionType.Exp


def build_nc(iters=1, variant=None):
    variant = variant or os.environ.get("KVARIANT", "full")
    bf16 = variant.startswith("bf16")
    MMDT = BF16 if bf16 else F32R      # dtype of matmul operands
    DRDT = F32 if bf16 else F32R       # dram dtype (f32->bf16 cast in DMA)
    nc = bacc.Bacc()
    hsT = nc.dram_tensor("hsT", [C, S], DRDT, kind="ExternalInput")
    wq = nc.dram_tensor("wq", [C, CH], DRDT, kind="ExternalInput")
    wk = nc.dram_tensor("wk", [C, CH], DRDT, kind="ExternalInput")
    wv = nc.dram_tensor("wv", [C, CH], DRDT, kind="ExternalInput")
    wak = nc.dram_tensor("wak", [C, CH], DRDT, kind="ExternalInput")
    wav = nc.dram_tensor("wav", [C, CH], DRDT, kind="ExternalInput")
    wo = nc.dram_tensor("wo", [CH, C], F32, kind="ExternalInput")
    extT = nc.dram_tensor("extT", [C, NAUP], DRDT, kind="ExternalInput")
    extzT = nc.dram_tensor("extzT", [C, NAUP], DRDT, kind="ExternalInput")
    outp = nc.dram_tensor("outp", [S, C], F32, kind="ExternalOutput")
    ld = nc.gpsimd if bf16 else nc.sync   # casting loads need SWDGE

    import contextlib
    with tile.TileContext(nc) as tc, \
         nc.allow_low_precision(reason="fp32r softmax reciprocals; bf16 PV/Wo"), \
         (tc.For_i(0, iters, 1) if iters > 1 else contextlib.nullcontext()):
        with tc.tile_pool(name="pers", bufs=1) as pers:
            qT = pers.tile([D, NH, S], MMDT, name="qT")
            kT = pers.tile([D, NH, S], MMDT, name="kT")
            # v with ones col at 96 (softmax denominator); cols 80:96 zero pad
            vaug = pers.tile([128, SC, NH, 97], BF16, name="vaug")
            wo_sb = pers.tile([D, NH, C], BF16, name="wo_sb")
            aukT = pers.tile([D, NH, NAUP], MMDT, name="aukT")
            ones80 = pers.tile([1, D], MMDT, name="ones80")
            auvaug = pers.tile([NAUP, NH, 98], MMDT, name="auvaug")

            nc.vector.memset(vaug[:, :, :, 80:96], 0.0)
            nc.vector.memset(vaug[:, :, :, 96:97], 1.0)
            # f32r memset is unsupported in codegen; stage via f32 + copy
            scr1 = pers.tile([1, D], F32, name="scr1")
            nc.vector.memset(scr1, 1.0)
            nc.vector.tensor_copy(ones80, scr1)
            # auvaug pad layout: [80:96]=0, [96]=ones (rows 0:13 only -- the
            # padded 14th key must not enter the softmax denominator), [97]=0
            scr2 = pers.tile([NAUP, NH, 18], F32, name="scr2")
            nc.vector.memset(scr2, 0.0)
            nc.vector.memset(scr2[0:13, :, 16:17], 1.0)
            nc.vector.tensor_copy(auvaug[:, :, 80:98], scr2)

            # ---------------- Phase B: projections ----------------
            with tc.tile_pool(name="projp", bufs=1) as projp, \
                 tc.tile_pool(name="wts", bufs=2) as wpool, \
                 tc.tile_pool(name="ppsum", bufs=4, space="PSUM") as pps:
                hsT_sb = projp.tile([128, KC, S], MMDT, name="hsT_sb")
                ld.dma_start(out=hsT_sb, in_=hsT[:].rearrange("(c p) s -> p c s", p=128))
                nc.gpsimd.dma_start(out=wo_sb, in_=wo[:].rearrange("(h d) n -> d h n", d=D))
                ext_sb = projp.tile([128, KC, NAUP], MMDT, name="ext_sb")
                ld.dma_start(out=ext_sb, in_=extT[:].rearrange("(c p) n -> p c n", p=128))
                extz_sb = projp.tile([128, KC, NAUP], MMDT, name="extz_sb")
                ld.dma_start(out=extz_sb, in_=extzT[:].rearrange("(c p) n -> p c n", p=128))

                # q and k projections, per head (transposed output)
                for wdram, dstT in ((wq, qT), (wk, kT)):
                    w_sb = wpool.tile([128, KC, CH], MMDT, tag="w", name="w_sb")
                    ld.dma_start(out=w_sb, in_=wdram[:].rearrange("(c p) n -> p c n", p=128))
                    for h in range(NH):
                        for nb in range(S // 512):
                            ps = pps.tile([D, 512], F32, tag="pp", name="ps_qk")
                            for c in range(KC):
                                nc.tensor.matmul(
                                    ps,
                                    w_sb[:, c, h * D:(h + 1) * D],
                                    hsT_sb[:, c, nb * 512:(nb + 1) * 512],
                                    start=(c == 0), stop=(c == KC - 1),
                                )
                            nc.vector.tensor_copy(dstT[:, h, nb * 512:(nb + 1) * 512], ps)

                # v projection (natural layout, bf16, strided into vaug)
                w_sb = wpool.tile([128, KC, CH], MMDT, tag="w", name="wv_sb")
                ld.dma_start(out=w_sb, in_=wv[:].rearrange("(c p) n -> p c n", p=128))
                for sc in range(SC):
                    ps = pps.tile([128, CH], F32, tag="pp", name="ps_v")
                    for c in range(KC):
                        nc.tensor.matmul(
                            ps,
                            hsT_sb[:, c, sc * 128:(sc + 1) * 128],
                            w_sb[:, c, :],
                            start=(c == 0), stop=(c == KC - 1),
                        )
                    nc.vector.tensor_copy(
                        vaug[:, sc, :, 0:80], ps.rearrange("p (h d) -> p h d", d=D)
                    )

                # au_k projection (transposed, per head)
                w_sb = wpool.tile([128, KC, CH], MMDT, tag="w", name="wak_sb")
                ld.dma_start(out=w_sb, in_=wak[:].rearrange("(c p) n -> p c n", p=128))
                for h in range(NH):
                    ps = pps.tile([D, NAUP], F32, tag="pp", name="ps_auk")
                    for c in range(KC):
                        nc.tensor.matmul(
                            ps,
                            w_sb[:, c, h * D:(h + 1) * D],
                            ext_sb[:, c, :],
                            start=(c == 0), stop=(c == KC - 1),
                        )
                    nc.vector.tensor_copy(aukT[:, h, :], ps)

                # au_v projection (natural [13, 320], gamma pre-folded on host)
                w_sb = wpool.tile([128, KC, CH], MMDT, tag="w", name="wav_sb")
                ld.dma_start(out=w_sb, in_=wav[:].rearrange("(c p) n -> p c n", p=128))
                ps = pps.tile([NAUP, CH], F32, tag="pp", name="ps_auv")
                for c in range(KC):
                    nc.tensor.matmul(
                        ps,
                        extz_sb[:, c, :],
                        w_sb[:, c, :],
                        start=(c == 0), stop=(c == KC - 1),
                    )
                nc.vector.tensor_copy(
                    auvaug[:, :, 0:80], ps.rearrange("p (h d) -> p h d", d=D)
                )

            # ---------------- Phase B2: AU branch attention ----------------
            with tc.tile_pool(name="aupers", bufs=1) as aupers:
                auout = aupers.tile([D, NH, S], BF16, name="auout")
                ausums = aupers.tile([1, NH, S], F32, name="ausums")
                with tc.tile_pool(name="aupsum", bufs=1, space="PSUM") as aups, \
                     tc.tile_pool(name="auexpp", bufs=2) as auexpp:
                    for h in range(NH):
                        aus = aups.tile([NAUP, S], F32, tag="aus", name="aus")
                        for nb in range(S // 512):
                            nc.tensor.matmul(
                                aus[:, nb * 512:(nb + 1) * 512],
                                aukT[:, h, :],
                                qT[:, h, nb * 512:(nb + 1) * 512],
                                start=True, stop=True,
                            )
                        au_e = auexpp.tile([NAUP, S], MMDT, tag="aue", name="au_e")
                        nc.scalar.activation(out=au_e, in_=aus, func=EXP)
                        auo = aups.tile([98, S], F32, tag="auo", name="auo")
                        for nb in range(S // 512):
                            nc.tensor.matmul(
                                auo[:, nb * 512:(nb + 1) * 512],
                                auvaug[:, h, :],
                                au_e[:, nb * 512:(nb + 1) * 512],
                                start=True, stop=True,
                            )
                        nc.vector.tensor_copy(auout[:, h, :], auo[0:80, :])
                        nc.vector.tensor_copy(ausums[0:1, h, :], auo[96:97, :])

                # ---------------- Phase C: main attention + merge + Wo ----------------
                with tc.tile_pool(name="spool", bufs=2, space="PSUM") as spool, \
                     tc.tile_pool(name="opool", bufs=2, space="PSUM") as opool, \
                     tc.tile_pool(name="expp", bufs=3) as expp, \
                     tc.tile_pool(name="mpool", bufs=2) as mpool, \
                     tc.tile_pool(name="scrp", bufs=1) as scrp, \
                     tc.tile_pool(name="outp_sb", bufs=3) as outsb_pool:
                    for qb in range(NQB):
                        q0 = qb * QB
                        merged = mpool.tile([D, NH, QB], BF16, tag="mg", name="merged")
                        for h in range(NH):
                            if variant == "noattn":
                                nc.vector.memset(merged[:, h, :], 0.001)
                                continue
                            outT = opool.tile([97, QB], F32, tag="ot", name="outT")
                            for kc in range(SC):
                                sco = spool.tile([128, QB], F32, tag="sc", name="sco")
                                for nn in range(QB // 512):
                                    nc.tensor.matmul(
                                        sco[:, nn * 512:(nn + 1) * 512],
                                        kT[:, h, kc * 128:(kc + 1) * 128],
                                        qT[:, h, q0 + nn * 512:q0 + (nn + 1) * 512],
                                        start=True, stop=True,
                                    )
                                ex = expp.tile([128, QB], BF16, tag="ex", name="ex")
                                if variant == "noexp":
                                    nc.vector.tensor_copy(ex, sco)
                                else:
                                    nc.scalar.activation(out=ex, in_=sco, func=EXP)
                                for nn in range(QB // 512):
                                    nc.tensor.matmul(
                                        outT[:, nn * 512:(nn + 1) * 512],
                                        vaug[:, kc, h, :],
                                        ex[:, nn * 512:(nn + 1) * 512],
                                        start=(kc == 0), stop=(kc == SC - 1),
                                    )
                            if variant in ("nomerge",):
                                nc.vector.tensor_copy(merged[:, h, :], outT[0:80, :])
                                continue
                            # softmax denominators -> PE broadcast -> merge with AU
                            rec_m = scrp.tile([1, QB], MMDT, tag="rm", name="rec_m")
                            nc.vector.reciprocal(rec_m, outT[96:97, :])
                            rec_a = scrp.tile([1, QB], MMDT, tag="ra", name="rec_a")
                            nc.vector.reciprocal(rec_a, ausums[0:1, h, q0:q0 + QB])
                            for rec, btag in ((rec_m, "bm"), (rec_a, "ba")):
                                bcp = spool.tile([D, QB], F32, tag="sc", name="bcp")
                                for nn in range(QB // 512):
                                    nc.tensor.matmul(
                                        bcp[:, nn * 512:(nn + 1) * 512],
                                        ones80,
                                        rec[:, nn * 512:(nn + 1) * 512],
                                        start=True, stop=True,
                                    )
                                bsb = scrp.tile([D, QB], F32, tag=btag, name="b_" + btag)
                                nc.vector.tensor_copy(bsb, bcp)
                                if btag == "bm":
                                    bc_m = bsb
                                else:
                                    bc_a = bsb
                            tmp_m = scrp.tile([D, QB], F32, tag="tm", name="tmp_m")
                            nc.vector.tensor_mul(tmp_m, outT[0:80, :], bc_m)
                            tmp_a = scrp.tile([D, QB], F32, tag="ta", name="tmp_a")
                            nc.vector.tensor_mul(tmp_a, auout[:, h, q0:q0 + QB], bc_a)
                            nc.vector.tensor_add(merged[:, h, :], tmp_m, tmp_a)

                        # Wo projection for this q-block (bf16)
                        for sj in range(QB // 128):
                            wo_ps = opool.tile([128, 2, 512], F32, tag="ot", name="wo_ps")
                            for nn in range(2):
                                for h in range(NH):
                                    nc.tensor.matmul(
                                        wo_ps[:, nn, 0:320],
                                        merged[:, h, sj * 128:(sj + 1) * 128],
                                        wo_sb[:, h, nn * 320:(nn + 1) * 320],
                                        start=(h == 0), stop=(h == NH - 1),
                                    )
                            o_sb = outsb_pool.tile([128, 2, 320], F32, tag="ob", name="o_sb")
                            nc.vector.tensor_copy(o_sb, wo_ps[:, :, 0:320])
                            s0 = q0 + sj * 128
                            nc.sync.dma_start(out=outp[s0:s0 + 128, :], in_=o_sb)
    nc.compile()
    return nc


_NC_CACHE = {}
LAST_EXEC_NS = None
LAST_RESULT = None


def _get_nc():
    if "nc" not in _NC_CACHE:
        _NC_CACHE["nc"] = build_nc()
    return _NC_CACHE["nc"]


def make_in_maps(inputs):
    hs = np.asarray(inputs["hidden_states"], np.float32)
    au = np.asarray(inputs["au_embedding"], np.float32)
    Wq = np.asarray(inputs["Wq"], np.float32)
    Wk = np.asarray(inputs["Wk"], np.float32)
    Wv = np.asarray(inputs["Wv"], np.float32)
    Wak = np.asarray(inputs["Wak"], np.float32)
    Wav = np.asarray(inputs["Wav"], np.float32)
    null_token = np.asarray(inputs["null_token"], np.float32).reshape(1, C)
    gamma = np.asarray(inputs["gamma"], np.float32)
    Wo = np.asarray(inputs["Wo"], np.float32)

    Wq_s = Wq * SCALE
    Wav_g = Wav * gamma[None, :]

    in_maps = []
    for c in range(8):
        b, hg = divmod(c, 2)
        sl = slice(hg * CH, (hg + 1) * CH)
        ext = np.concatenate(
            [au[b], null_token, np.zeros((1, C), np.float32)], axis=0
        )  # [14, C]; row 13 is fp32r even-size padding
        extz = ext.copy()
        extz[NAU - 1] = 0.0
        in_maps.append({
            "hsT": np.ascontiguousarray(hs[b].T),
            "wq": np.ascontiguousarray(Wq_s[:, sl]),
            "wk": np.ascontiguousarray(Wk[:, sl]),
            "wv": np.ascontiguousarray(Wv[:, sl]),
            "wak": np.ascontiguousarray(Wak[:, sl]),
            "wav": np.ascontiguousarray(Wav_g[:, sl]),
            "wo": np.ascontiguousarray(Wo[sl, :]),
            "extT": np.ascontiguousarray(ext.T),
            "extzT": np.ascontiguousarray(extz.T),
        })
    return in_maps


def kernel(**inputs):
    global LAST_EXEC_NS, LAST_RESULT
    hs = np.asarray(inputs["hidden_states"], np.float32)
    bo = np.asarray(inputs["bo"], np.float32)
    in_maps = make_in_maps(inputs)
    nc = _get_nc()
    trace = os.environ.get("KERNEL_TRACE", "0") == "1"
    res = run_bass_kernel_spmd(nc, in_maps, list(range(8)), trace=trace)
    LAST_EXEC_NS = res.exec_time_ns
    LAST_RESULT = res
    out = np.empty((B, S, C), np.float32)
    for b in range(B):
        out[b] = res.results[2 * b]["outp"] + res.results[2 * b + 1]["outp"]
        out[b] += bo[None, :]
        out[b] += hs[b]
    return out



# revision 11
# speedup vs baseline: 1.3325x; 1.3325x over previous
"""Trainium2 Bass kernel for nn_AUAttnProcessor (AU-token attention processor).

Sharding: 8 cores = (batch b, head-group hg). Core c handles batch c//2 and
heads [4*(c%2), 4*(c%2)+4) (Ch=320 of C=640 channels).  Wq/Wk/Wv/Wak/Wav are
column-sharded, Wo row-sharded; each core emits a partial [S, C] output and the
host reduces the two partials per batch and adds bias + residual.

Per-core pipeline (transposed flash-attention orientation, bf16 operands,
inputs pre-cast to bf16 on the host):
  qT/kT = (W.T @ hsT)            [80, S] per head, evacuated bf16
  vaug  = hs @ Wv  + ones col 80 [128, sc, h, 82] bf16
  scoresT[kc] = kT_chunk.T @ qT  -> PSUM f32 [128, 1024]
  expT = Exp(scoresT) (ACT)      -> SBUF bf16
  outT += vaug_chunk.T @ expT    [82, 1024] PSUM; row 80 = softmax denominator
  raw_m = cast(outT) bf16        [82, NH, S] (row 80 = denominators)
  denominators: cast-DMA rows -> dsum[4,S] f32 -> reciprocal_approx_fast
                -> cast-DMA -> rrows [1, 2, NH, S] bf16 (partition 0)
  bc = partition_broadcast(rrows row)  (GpSimd, SBUF bf16)
  osb = raw * bc  (DVE bf16 2x mode)   per head and branch
  Wo: accumulate 16 matmuls (main+au, 4 heads) per 128-query chunk -> [128, 640]
"""

import os
import sys

import numpy as np

for _p in ("/opt/trn_rl_repo",):
    if os.path.isdir(_p) and _p not in sys.path:
        sys.path.insert(0, _p)

import ml_dtypes

import concourse.bass as bass
import concourse.tile as tile
from concourse import bacc, mybir
from concourse.bass_utils import run_bass_kernel_spmd

# Problem dims
B, S, C, H, D = 4, 2048, 640, 8, 80
NH = 4            # heads per core
CH = NH * D       # 320 channels per core
KC = C // 128     # 5 contraction chunks
SC = S // 128     # 16 sequence chunks
NAU = 13          # 12 AU tokens + 1 null token
NAUP = 14         # padded to even
VA = 82           # vaug rows: 80 v + ones row (80) + zero pad (81)
QB = 1024         # q-block width
NQB = S // QB
SCALE = float(D) ** -0.5

F32 = mybir.dt.float32
BF16 = mybir.dt.bfloat16
EXP = mybir.ActivationFunctionType.Exp
BF16_NP = ml_dtypes.bfloat16


def _phase_p(nc, tc, pers_tiles, raw_au, dram):
    """Projections + AU attention. Closes its pools on exit (frees hsT_sb)."""
    qT, kT, vaug, wo_sb, aukT, auvaug = pers_tiles
    hsT, wq, wk, wv, wak, wav, wo, extT, extzT = dram
    with tc.tile_pool(name="projp", bufs=1) as projp, \
         tc.tile_pool(name="wts", bufs=2) as wpool, \
         tc.tile_pool(name="ppsum", bufs=4, space="PSUM") as pps, \
         tc.tile_pool(name="aupsum", bufs=1, space="PSUM") as aups, \
         tc.tile_pool(name="auexpp", bufs=2) as auexpp:
        ext_sb = projp.tile([128, KC, NAUP], BF16, name="ext_sb")
        nc.sync.dma_start(
            out=ext_sb, in_=extT[:].rearrange("(c p) n -> p c n", p=128))
        extz_sb = projp.tile([128, KC, NAUP], BF16, name="extz_sb")
        nc.sync.dma_start(
            out=extz_sb, in_=extzT[:].rearrange("(c p) n -> p c n", p=128))
        hsT_sb = projp.tile([128, KC, S], BF16, name="hsT_sb")
        nc.sync.dma_start(
            out=hsT_sb, in_=hsT[:].rearrange("(c p) s -> p c s", p=128))
        nc.sync.dma_start(
            out=wo_sb, in_=wo[:].rearrange("(h d) n -> d h n", d=D))

        # au_k projection first: tiny matmuls warm the PE while the hsT DMA
        # is in flight
        w_sb = wpool.tile([128, KC, CH], BF16, tag="w", name="wak_sb")
        nc.sync.dma_start(
            out=w_sb, in_=wak[:].rearrange("(c p) n -> p c n", p=128))
        for h in range(NH):
            ps = pps.tile([D, NAUP], F32, tag="pp", name="ps_auk")
            for c in range(KC):
                nc.tensor.matmul(
                    ps,
                    w_sb[:, c, h * D:(h + 1) * D],
                    ext_sb[:, c, :],
                    start=(c == 0), stop=(c == KC - 1),
                )
            nc.vector.tensor_copy(aukT[:, h, :], ps)

        # au_v projection (natural [14, 320], gamma pre-folded on host)
        w_sb = wpool.tile([128, KC, CH], BF16, tag="w", name="wav_sb")
        nc.sync.dma_start(
            out=w_sb, in_=wav[:].rearrange("(c p) n -> p c n", p=128))
        ps = pps.tile([NAUP, CH], F32, tag="pp", name="ps_auv")
        for c in range(KC):
            nc.tensor.matmul(
                ps,
                extz_sb[:, c, :],
                w_sb[:, c, :],
                start=(c == 0), stop=(c == KC - 1),
            )
        nc.vector.tensor_copy(
            auvaug[:, :, 0:80], ps.rearrange("p (h d) -> p h d", d=D))

        # q and k projections, per head (transposed output, bf16)
        for wdram, dstT, evac in ((wq, qT, "act"), (wk, kT, "dve")):
            w_sb = wpool.tile([128, KC, CH], BF16, tag="w", name="w_sb")
            nc.sync.dma_start(
                out=w_sb, in_=wdram[:].rearrange("(c p) n -> p c n", p=128))
            for h in range(NH):
                for nb in range(S // 512):
                    ps = pps.tile([D, 512], F32, tag="pp", name="ps_qk")
                    for c in range(KC):
                        nc.tensor.matmul(
                            ps,
                            w_sb[:, c, h * D:(h + 1) * D],
                            hsT_sb[:, c, nb * 512:(nb + 1) * 512],
                            start=(c == 0), stop=(c == KC - 1),
                        )
                    if evac == "act":
                        nc.scalar.copy(dstT[:, h, nb * 512:(nb + 1) * 512], ps)
                    else:
                        nc.vector.tensor_copy(
                            dstT[:, h, nb * 512:(nb + 1) * 512], ps)

        # v projection (natural layout, strided into vaug)
        w_sb = wpool.tile([128, KC, CH], BF16, tag="w", name="wv_sb")
        nc.sync.dma_start(
            out=w_sb, in_=wv[:].rearrange("(c p) n -> p c n", p=128))
        for sc in range(SC):
            ps = pps.tile([128, CH], F32, tag="pp", name="ps_v")
            for c in range(KC):
                nc.tensor.matmul(
                    ps,
                    hsT_sb[:, c, sc * 128:(sc + 1) * 128],
                    w_sb[:, c, :],
                    start=(c == 0), stop=(c == KC - 1),
                )
            nc.vector.tensor_copy(
                vaug[:, sc, :, 0:80], ps.rearrange("p (h d) -> p h d", d=D))

        # AU branch attention (chunked along S so its PSUM pool coexists
        # with the projection pool)
        for h in range(NH):
            for hf in range(2):
                f0 = hf * 1024
                aus = aups.tile([NAUP, 1024], F32, tag="aus", name="aus")
                for nb in range(2):
                    nc.tensor.matmul(
                        aus[:, nb * 512:(nb + 1) * 512],
                        aukT[:, h, :],
                        qT[:, h, f0 + nb * 512:f0 + (nb + 1) * 512],
                        start=True, stop=True,
                    )
                au_e = auexpp.tile([NAUP, 1024], BF16, tag="aue", name="au_e")
                nc.scalar.activation(out=au_e, in_=aus, func=EXP)
                auo = aups.tile([VA, 1024], F32, tag="auo", name="auo")
                for nb in range(2):
                    nc.tensor.matmul(
                        auo[:, nb * 512:(nb + 1) * 512],
                        auvaug[:, h, :],
                        au_e[:, nb * 512:(nb + 1) * 512],
                        start=True, stop=True,
                    )
                nc.vector.tensor_copy(raw_au[:, h, f0:f0 + 1024], auo)


def build_nc():
    nc = bacc.Bacc()
    hsT = nc.dram_tensor("hsT", [C, S], BF16, kind="ExternalInput")
    wq = nc.dram_tensor("wq", [C, CH], BF16, kind="ExternalInput")
    wk = nc.dram_tensor("wk", [C, CH], BF16, kind="ExternalInput")
    wv = nc.dram_tensor("wv", [C, CH], BF16, kind="ExternalInput")
    wak = nc.dram_tensor("wak", [C, CH], BF16, kind="ExternalInput")
    wav = nc.dram_tensor("wav", [C, CH], BF16, kind="ExternalInput")
    wo = nc.dram_tensor("wo", [CH, C], BF16, kind="ExternalInput")
    extT = nc.dram_tensor("extT", [C, NAUP], BF16, kind="ExternalInput")
    extzT = nc.dram_tensor("extzT", [C, NAUP], BF16, kind="ExternalInput")
    outp = nc.dram_tensor("outp", [S, C], F32, kind="ExternalOutput")
    dram = (hsT, wq, wk, wv, wak, wav, wo, extT, extzT)

    with tile.TileContext(nc) as tc, \
         nc.allow_low_precision(reason="bf16 attention pipeline; approx recip"):
        with tc.tile_pool(name="pers", bufs=1) as pers:
            qT = pers.tile([D, NH, S], BF16, name="qT")
            kT = pers.tile([D, NH, S], BF16, name="kT")
            vaug = pers.tile([128, SC, NH, VA], BF16, name="vaug")
            wo_sb = pers.tile([D, NH, C], BF16, name="wo_sb")
            aukT = pers.tile([D, NH, NAUP], BF16, name="aukT")
            auvaug = pers.tile([NAUP, NH, VA], BF16, name="auvaug")

            nc.vector.memset(vaug[:, :, :, 80:81], 1.0)
            nc.vector.memset(vaug[:, :, :, 81:82], 0.0)
            # auvaug: col 80 = 1 on rows 0:13 only (pad row 13 excluded from
            # the denominator), col 81 = 0
            nc.vector.memset(auvaug[:, :, 80:82], 0.0)
            nc.vector.memset(auvaug[0:NAU, :, 80:81], 1.0)

            with tc.tile_pool(name="pmid", bufs=1) as pmid:
                raw_au = pmid.tile([VA, NH, S], BF16, name="raw_au")

                _phase_p(nc, tc, (qT, kT, vaug, wo_sb, aukT, auvaug),
                         raw_au, dram)

                # Persistent tiles for phases B/C (allocated after the
                # projection pool closed, reusing hsT_sb's space)
                with tc.tile_pool(name="pers2", bufs=1) as pers2, \
                     tc.tile_pool(name="bcp", bufs=3) as bcp:
                    raw_m = pers2.tile([VA, NH, S], BF16, name="raw_m")
                    osb_m = pers2.tile([D, NH, S], BF16, name="osb_m")
                    osb_a = pers2.tile([D, NH, S], BF16, name="osb_a")
                    dsum_m = pers2.tile([4, S], F32, name="dsum_m")
                    dsum_a = pers2.tile([4, S], F32, name="dsum_a")
                    dsum_rm = pers2.tile([4, S], F32, name="dsum_rm")
                    dsum_ra = pers2.tile([4, S], F32, name="dsum_ra")
                    rrows = pers2.tile([1, 2, NH, S], BF16, name="rrows")

                    # au denominator pipeline + merge (overlaps early phase B)
                    nc.gpsimd.dma_start(out=dsum_a, in_=raw_au[80:81, :, :])
                    nc.vector.reciprocal_approx_fast(out=dsum_ra, in_=dsum_a)
                    nc.gpsimd.dma_start(out=rrows[0:1, 1, :, :], in_=dsum_ra)
                    for h in range(NH):
                        bc = bcp.tile([D, S], BF16, tag="bca", name="bc_a")
                        nc.gpsimd.partition_broadcast(bc, rrows[0:1, 1, h, :])
                        nc.vector.tensor_mul(
                            osb_a[:, h, :], raw_au[0:80, h, :], bc)

                    # ---------------- Phase B: main attention ----------------
                    with tc.tile_pool(name="spool", bufs=2, space="PSUM") as spool, \
                         tc.tile_pool(name="opool", bufs=2, space="PSUM") as opool, \
                         tc.tile_pool(name="expp", bufs=3) as expp:
                        for qb in range(NQB):
                            q0 = qb * QB
                            for h in range(NH):
                                outT = opool.tile(
                                    [VA, QB], F32, tag="ot", name="outT")
                                for kc in range(SC):
                                    sco = spool.tile(
                                        [128, QB], F32, tag="sc", name="sco")
                                    for nn in range(QB // 512):
                                        nc.tensor.matmul(
                                            sco[:, nn * 512:(nn + 1) * 512],
                                            kT[:, h, kc * 128:(kc + 1) * 128],
                                            qT[:, h,
                                               q0 + nn * 512:q0 + (nn + 1) * 512],
                                            start=True, stop=True,
                                        )
                                    ex = expp.tile(
                                        [128, QB], BF16, tag="ex", name="ex")
                                    nc.scalar.activation(
                                        out=ex, in_=sco, func=EXP)
                                    for nn in range(QB // 512):
                                        nc.tensor.matmul(
                                            outT[:, nn * 512:(nn + 1) * 512],
                                            vaug[:, kc, h, :],
                                            ex[:, nn * 512:(nn + 1) * 512],
                                            start=(kc == 0), stop=(kc == SC - 1),
                                        )
                                nc.vector.tensor_copy(
                                    raw_m[:, h, q0:q0 + QB], outT)

                            # denominator pipeline + merge for this q-block
                            # (overlaps the next q-block's attention)
                            nc.gpsimd.dma_start(
                                out=dsum_m[:, q0:q0 + QB],
                                in_=raw_m[80:81, :, q0:q0 + QB],
                            )
                            nc.vector.reciprocal_approx_fast(
                                out=dsum_rm[:, q0:q0 + QB],
                                in_=dsum_m[:, q0:q0 + QB])
                            nc.gpsimd.dma_start(
                                out=rrows[0:1, 0, :, q0:q0 + QB],
                                in_=dsum_rm[:, q0:q0 + QB],
                            )
                            for h in range(NH):
                                bc = bcp.tile(
                                    [D, QB], BF16, tag="bcm", name="bc_m")
                                nc.gpsimd.partition_broadcast(
                                    bc, rrows[0:1, 0, h, q0:q0 + QB])
                                nc.vector.tensor_mul(
                                    osb_m[:, h, q0:q0 + QB],
                                    raw_m[0:80, h, q0:q0 + QB],
                                    bc,
                                )

                    # ---------------- Phase C: Wo projection ----------------
                    with tc.tile_pool(name="wopool", bufs=3, space="PSUM") as wopool, \
                         tc.tile_pool(name="outp_sb", bufs=3) as outsb_pool:
                        for sj in range(S // 128):
                            s0 = sj * 128
                            wo_ps = wopool.tile(
                                [128, C], F32, tag="wo", name="wo_ps")
                            for n0, n1 in ((0, 512), (512, 640)):
                                k = 0
                                for osb in (osb_m, osb_a):
                                    for h in range(NH):
                                        nc.tensor.matmul(
                                            wo_ps[:, n0:n1],
                                            osb[:, h, s0:s0 + 128],
                                            wo_sb[:, h, n0:n1],
                                            start=(k == 0), stop=(k == 7),
                                        )
                                        k += 1
                            o_sb = outsb_pool.tile(
                                [128, C], F32, tag="ob", name="o_sb")
                            nc.scalar.copy(o_sb, wo_ps)
                            nc.sync.dma_start(
                                out=outp[s0:s0 + 128, :], in_=o_sb)
    nc.compile()
    return nc


_NC_CACHE = {}
LAST_EXEC_NS = None
LAST_RESULT = None


def _get_nc():
    if "nc" not in _NC_CACHE:
        _NC_CACHE["nc"] = build_nc()
    return _NC_CACHE["nc"]


def make_in_maps(inputs):
    hs = np.asarray(inputs["hidden_states"], np.float32)
    au = np.asarray(inputs["au_embedding"], np.float32)
    Wq = np.asarray(inputs["Wq"], np.float32)
    Wk = np.asarray(inputs["Wk"], np.float32)
    Wv = np.asarray(inputs["Wv"], np.float32)
    Wak = np.asarray(inputs["Wak"], np.float32)
    Wav = np.asarray(inputs["Wav"], np.float32)
    null_token = np.asarray(inputs["null_token"], np.float32).reshape(1, C)
    gamma = np.asarray(inputs["gamma"], np.float32)
    Wo = np.asarray(inputs["Wo"], np.float32)

    Wq_s = Wq * SCALE
    Wav_g = Wav * gamma[None, :]

    def b16(x):
        return np.ascontiguousarray(x.astype(BF16_NP))

    in_maps = []
    for c in range(8):
        b, hg = divmod(c, 2)
        sl = slice(hg * CH, (hg + 1) * CH)
        ext = np.concatenate(
            [au[b], null_token, np.zeros((1, C), np.float32)], axis=0
        )  # [14, C]; row 13 is even-size padding
        extz = ext.copy()
        extz[NAU - 1] = 0.0
        in_maps.append({
            "hsT": b16(hs[b].T),
            "wq": b16(Wq_s[:, sl]),
            "wk": b16(Wk[:, sl]),
            "wv": b16(Wv[:, sl]),
            "wak": b16(Wak[:, sl]),
            "wav": b16(Wav_g[:, sl]),
            "wo": b16(Wo[sl, :]),
            "extT": b16(ext.T),
            "extzT": b16(extz.T),
        })
    return in_maps


def kernel(**inputs):
    global LAST_EXEC_NS, LAST_RESULT
    hs = np.asarray(inputs["hidden_states"], np.float32)
    bo = np.asarray(inputs["bo"], np.float32)
    in_maps = make_in_maps(inputs)
    nc = _get_nc()
    trace = os.environ.get("KERNEL_TRACE", "0") == "1"
    res = run_bass_kernel_spmd(nc, in_maps, list(range(8)), trace=trace)
    LAST_EXEC_NS = res.exec_time_ns
    LAST_RESULT = res
    out = np.empty((B, S, C), np.float32)
    for b in range(B):
        out[b] = res.results[2 * b]["outp"] + res.results[2 * b + 1]["outp"]
        out[b] += bo[None, :]
        out[b] += hs[b]
    return out


# revision 12
# speedup vs baseline: 1.3640x; 1.0236x over previous
"""Trainium2 Bass kernel for nn_AUAttnProcessor (AU-token attention processor).

Sharding: 8 cores = (batch b, head-group hg). Core c handles batch c//2 and
heads [4*(c%2), 4*(c%2)+4) (Ch=320 of C=640 channels).  Wq/Wk/Wv/Wak/Wav are
column-sharded, Wo row-sharded; each core emits a partial [S, C] output and the
host reduces the two partials per batch and adds bias + residual.

Per-core pipeline (transposed flash-attention orientation, bf16 operands,
inputs pre-cast to bf16 on the host):
  qT/kT = (W.T @ hsT)            [80, S] per head, evacuated bf16
  vaug  = hs @ Wv  + ones col 80 [128, sc, h, 82] bf16
  scoresT[kc] = kT_chunk.T @ qT  -> PSUM f32 [128, 1024]
  expT = Exp(scoresT) (ACT)      -> SBUF bf16
  outT += vaug_chunk.T @ expT    [82, 1024] PSUM; row 80 = softmax denominator
  raw_m = cast(outT) bf16        [82, NH, S] (row 80 = denominators)
  denominators: cast-DMA rows -> dsum[4,S] f32 -> reciprocal_approx_fast
                -> cast-DMA -> rrows [1, 2, NH, S] bf16 (partition 0)
  bc = partition_broadcast(rrows row)  (GpSimd, SBUF bf16)
  osb = raw * bc  (DVE bf16 2x mode)   per head and branch
  Wo: accumulate 16 matmuls (main+au, 4 heads) per 128-query chunk -> [128, 640]
"""

import os
import sys

import numpy as np

for _p in ("/opt/trn_rl_repo",):
    if os.path.isdir(_p) and _p not in sys.path:
        sys.path.insert(0, _p)

import ml_dtypes

import concourse.bass as bass
import concourse.tile as tile
from concourse import bacc, mybir
from concourse.bass_utils import run_bass_kernel_spmd

# Problem dims
B, S, C, H, D = 4, 2048, 640, 8, 80
NH = 4            # heads per core
CH = NH * D       # 320 channels per core
KC = C // 128     # 5 contraction chunks
SC = S // 128     # 16 sequence chunks
NAU = 13          # 12 AU tokens + 1 null token
NAUP = 14         # padded to even
VA = 82           # vaug rows: 80 v + ones row (80) + zero pad (81)
QB = 1024         # q-block width
NQB = S // QB
SCALE = float(D) ** -0.5

F32 = mybir.dt.float32
BF16 = mybir.dt.bfloat16
EXP = mybir.ActivationFunctionType.Exp
BF16_NP = ml_dtypes.bfloat16


def _phase_p(nc, tc, pers_tiles, raw_au, dram):
    """Projections + AU attention. Closes its pools on exit (frees hsT_sb)."""
    qT, kT, vaug, wo_sb, aukT, auvaug = pers_tiles
    hsT, wq, wk, wv, wak, wav, wo, extT, extzT = dram
    with tc.tile_pool(name="projp", bufs=1) as projp, \
         tc.tile_pool(name="wts", bufs=2) as wpool, \
         tc.tile_pool(name="ppsum", bufs=4, space="PSUM") as pps:
        ext_sb = projp.tile([128, KC, NAUP], BF16, name="ext_sb")
        nc.sync.dma_start(
            out=ext_sb, in_=extT[:].rearrange("(c p) n -> p c n", p=128))
        extz_sb = projp.tile([128, KC, NAUP], BF16, name="extz_sb")
        nc.sync.dma_start(
            out=extz_sb, in_=extzT[:].rearrange("(c p) n -> p c n", p=128))
        hsT_sb = projp.tile([128, KC, S], BF16, name="hsT_sb")
        nc.sync.dma_start(
            out=hsT_sb, in_=hsT[:].rearrange("(c p) s -> p c s", p=128))
        nc.sync.dma_start(
            out=wo_sb, in_=wo[:].rearrange("(h d) n -> d h n", d=D))

        # au_k projection first: tiny matmuls warm the PE while the hsT DMA
        # is in flight
        w_sb = wpool.tile([128, KC, CH], BF16, tag="w", name="wak_sb")
        nc.sync.dma_start(
            out=w_sb, in_=wak[:].rearrange("(c p) n -> p c n", p=128))
        for h in range(NH):
            ps = pps.tile([D, NAUP], F32, tag="pp", name="ps_auk")
            for c in range(KC):
                nc.tensor.matmul(
                    ps,
                    w_sb[:, c, h * D:(h + 1) * D],
                    ext_sb[:, c, :],
                    start=(c == 0), stop=(c == KC - 1),
                )
            nc.vector.tensor_copy(aukT[:, h, :], ps)

        # au_v projection (natural [14, 320], gamma pre-folded on host)
        w_sb = wpool.tile([128, KC, CH], BF16, tag="w", name="wav_sb")
        nc.sync.dma_start(
            out=w_sb, in_=wav[:].rearrange("(c p) n -> p c n", p=128))
        ps = pps.tile([NAUP, CH], F32, tag="pp", name="ps_auv")
        for c in range(KC):
            nc.tensor.matmul(
                ps,
                extz_sb[:, c, :],
                w_sb[:, c, :],
                start=(c == 0), stop=(c == KC - 1),
            )
        nc.vector.tensor_copy(
            auvaug[:, :, 0:80], ps.rearrange("p (h d) -> p h d", d=D))

        # q and k projections, per head (transposed output, bf16)
        for wdram, dstT, evac in ((wq, qT, "act"), (wk, kT, "dve")):
            w_sb = wpool.tile([128, KC, CH], BF16, tag="w", name="w_sb")
            nc.sync.dma_start(
                out=w_sb, in_=wdram[:].rearrange("(c p) n -> p c n", p=128))
            for h in range(NH):
                for nb in range(S // 512):
                    ps = pps.tile([D, 512], F32, tag="pp", name="ps_qk")
                    for c in range(KC):
                        nc.tensor.matmul(
                            ps,
                            w_sb[:, c, h * D:(h + 1) * D],
                            hsT_sb[:, c, nb * 512:(nb + 1) * 512],
                            start=(c == 0), stop=(c == KC - 1),
                        )
                    if evac == "act":
                        nc.scalar.copy(dstT[:, h, nb * 512:(nb + 1) * 512], ps)
                    else:
                        nc.vector.tensor_copy(
                            dstT[:, h, nb * 512:(nb + 1) * 512], ps)

        # v projection (natural layout, strided into vaug)
        w_sb = wpool.tile([128, KC, CH], BF16, tag="w", name="wv_sb")
        nc.sync.dma_start(
            out=w_sb, in_=wv[:].rearrange("(c p) n -> p c n", p=128))
        for sc in range(SC):
            ps = pps.tile([128, CH], F32, tag="pp", name="ps_v")
            for c in range(KC):
                nc.tensor.matmul(
                    ps,
                    hsT_sb[:, c, sc * 128:(sc + 1) * 128],
                    w_sb[:, c, :],
                    start=(c == 0), stop=(c == KC - 1),
                )
            nc.vector.tensor_copy(
                vaug[:, sc, :, 0:80], ps.rearrange("p (h d) -> p h d", d=D))


def build_nc():
    nc = bacc.Bacc()
    hsT = nc.dram_tensor("hsT", [C, S], BF16, kind="ExternalInput")
    wq = nc.dram_tensor("wq", [C, CH], BF16, kind="ExternalInput")
    wk = nc.dram_tensor("wk", [C, CH], BF16, kind="ExternalInput")
    wv = nc.dram_tensor("wv", [C, CH], BF16, kind="ExternalInput")
    wak = nc.dram_tensor("wak", [C, CH], BF16, kind="ExternalInput")
    wav = nc.dram_tensor("wav", [C, CH], BF16, kind="ExternalInput")
    wo = nc.dram_tensor("wo", [CH, C], BF16, kind="ExternalInput")
    extT = nc.dram_tensor("extT", [C, NAUP], BF16, kind="ExternalInput")
    extzT = nc.dram_tensor("extzT", [C, NAUP], BF16, kind="ExternalInput")
    outp = nc.dram_tensor("outp", [S, C], F32, kind="ExternalOutput")
    dram = (hsT, wq, wk, wv, wak, wav, wo, extT, extzT)

    with tile.TileContext(nc) as tc, \
         nc.allow_low_precision(reason="bf16 attention pipeline; approx recip"):
        with tc.tile_pool(name="pers", bufs=1) as pers:
            qT = pers.tile([D, NH, S], BF16, name="qT")
            kT = pers.tile([D, NH, S], BF16, name="kT")
            vaug = pers.tile([128, SC, NH, VA], BF16, name="vaug")
            wo_sb = pers.tile([D, NH, C], BF16, name="wo_sb")
            aukT = pers.tile([D, NH, NAUP], BF16, name="aukT")
            auvaug = pers.tile([NAUP, NH, VA], BF16, name="auvaug")

            nc.vector.memset(vaug[:, :, :, 80:81], 1.0)
            nc.vector.memset(vaug[:, :, :, 81:82], 0.0)
            # auvaug: col 80 = 1 on rows 0:13 only (pad row 13 excluded from
            # the denominator), col 81 = 0
            nc.vector.memset(auvaug[:, :, 80:82], 0.0)
            nc.vector.memset(auvaug[0:NAU, :, 80:81], 1.0)

            with tc.tile_pool(name="pmid", bufs=1) as pmid:
                raw_au = pmid.tile([VA, NH, S], BF16, name="raw_au")

                _phase_p(nc, tc, (qT, kT, vaug, wo_sb, aukT, auvaug),
                         raw_au, dram)

                # Persistent tiles for phases B/C (allocated after the
                # projection pool closed, reusing hsT_sb's space)
                with tc.tile_pool(name="pers2", bufs=1) as pers2, \
                     tc.tile_pool(name="bcp", bufs=3) as bcp:
                    raw_m = pers2.tile([VA, NH, S], BF16, name="raw_m")
                    osb_m = pers2.tile([D, NH, S], BF16, name="osb_m")
                    osb_a = pers2.tile([D, NH, S], BF16, name="osb_a")
                    dsum_m = pers2.tile([4, S], F32, name="dsum_m")
                    dsum_a = pers2.tile([4, S], F32, name="dsum_a")
                    dsum_rm = pers2.tile([4, S], F32, name="dsum_rm")
                    dsum_ra = pers2.tile([4, S], F32, name="dsum_ra")
                    rrows = pers2.tile([1, 2, NH, S], BF16, name="rrows")

                    # ---------------- Phase B: main attention ----------------
                    with tc.tile_pool(name="spool", bufs=2, space="PSUM") as spool, \
                         tc.tile_pool(name="opool", bufs=2, space="PSUM") as opool, \
                         tc.tile_pool(name="expp", bufs=3) as expp:
                        # AU branch attention first, inside the same pools
                        # (no PSUM pool transition before the main loop)
                        for h in range(NH):
                            for hf in range(2):
                                f0 = hf * 1024
                                aus = spool.tile(
                                    [128, QB], F32, tag="sc", name="aus")
                                for nb in range(2):
                                    nc.tensor.matmul(
                                        aus[0:NAUP, nb * 512:(nb + 1) * 512],
                                        aukT[:, h, :],
                                        qT[:, h, f0 + nb * 512:f0 + (nb + 1) * 512],
                                        start=True, stop=True,
                                    )
                                au_e = expp.tile(
                                    [128, QB], BF16, tag="ex", name="au_e")
                                nc.scalar.activation(
                                    out=au_e[0:NAUP, :], in_=aus[0:NAUP, :],
                                    func=EXP)
                                auo = opool.tile(
                                    [VA, QB], F32, tag="ot", name="auo")
                                for nb in range(2):
                                    nc.tensor.matmul(
                                        auo[:, nb * 512:(nb + 1) * 512],
                                        auvaug[:, h, :],
                                        au_e[0:NAUP, nb * 512:(nb + 1) * 512],
                                        start=True, stop=True,
                                    )
                                nc.vector.tensor_copy(
                                    raw_au[:, h, f0:f0 + 1024], auo)

                        # au denominator pipeline + merge (overlaps early
                        # main attention)
                        nc.gpsimd.dma_start(out=dsum_a, in_=raw_au[80:81, :, :])
                        nc.vector.reciprocal_approx_fast(
                            out=dsum_ra, in_=dsum_a)
                        nc.gpsimd.dma_start(
                            out=rrows[0:1, 1, :, :], in_=dsum_ra)
                        for h in range(NH):
                            bc = bcp.tile([D, S], BF16, tag="bca", name="bc_a")
                            nc.gpsimd.partition_broadcast(
                                bc, rrows[0:1, 1, h, :])
                            nc.vector.tensor_mul(
                                osb_a[:, h, :], raw_au[0:80, h, :], bc)

                        for qb in range(NQB):
                            q0 = qb * QB
                            for h in range(NH):
                                outT = opool.tile(
                                    [VA, QB], F32, tag="ot", name="outT")
                                for kc in range(SC):
                                    sco = spool.tile(
                                        [128, QB], F32, tag="sc", name="sco")
                                    for nn in range(QB // 512):
                                        nc.tensor.matmul(
                                            sco[:, nn * 512:(nn + 1) * 512],
                                            kT[:, h, kc * 128:(kc + 1) * 128],
                                            qT[:, h,
                                               q0 + nn * 512:q0 + (nn + 1) * 512],
                                            start=True, stop=True,
                                        )
                                    ex = expp.tile(
                                        [128, QB], BF16, tag="ex", name="ex")
                                    nc.scalar.activation(
                                        out=ex, in_=sco, func=EXP)
                                    for nn in range(QB // 512):
                                        nc.tensor.matmul(
                                            outT[:, nn * 512:(nn + 1) * 512],
                                            vaug[:, kc, h, :],
                                            ex[:, nn * 512:(nn + 1) * 512],
                                            start=(kc == 0), stop=(kc == SC - 1),
                                        )
                                nc.vector.tensor_copy(
                                    raw_m[:, h, q0:q0 + QB], outT)

                            # denominator pipeline + merge for this q-block
                            # (overlaps the next q-block's attention)
                            nc.gpsimd.dma_start(
                                out=dsum_m[:, q0:q0 + QB],
                                in_=raw_m[80:81, :, q0:q0 + QB],
                            )
                            nc.vector.reciprocal_approx_fast(
                                out=dsum_rm[:, q0:q0 + QB],
                                in_=dsum_m[:, q0:q0 + QB])
                            nc.gpsimd.dma_start(
                                out=rrows[0:1, 0, :, q0:q0 + QB],
                                in_=dsum_rm[:, q0:q0 + QB],
                            )
                            for h in range(NH):
                                bc = bcp.tile(
                                    [D, QB], BF16, tag="bcm", name="bc_m")
                                nc.gpsimd.partition_broadcast(
                                    bc, rrows[0:1, 0, h, q0:q0 + QB])
                                nc.vector.tensor_mul(
                                    osb_m[:, h, q0:q0 + QB],
                                    raw_m[0:80, h, q0:q0 + QB],
                                    bc,
                                )

                    # ---------------- Phase C: Wo projection ----------------
                    with tc.tile_pool(name="wopool", bufs=3, space="PSUM") as wopool, \
                         tc.tile_pool(name="outp_sb", bufs=3) as outsb_pool:
                        for sj in range(S // 128):
                            s0 = sj * 128
                            wo_ps = wopool.tile(
                                [128, C], F32, tag="wo", name="wo_ps")
                            for n0, n1 in ((0, 512), (512, 640)):
                                k = 0
                                for osb in (osb_m, osb_a):
                                    for h in range(NH):
                                        nc.tensor.matmul(
                                            wo_ps[:, n0:n1],
                                            osb[:, h, s0:s0 + 128],
                                            wo_sb[:, h, n0:n1],
                                            start=(k == 0), stop=(k == 7),
                                        )
                                        k += 1
                            o_sb = outsb_pool.tile(
                                [128, C], F32, tag="ob", name="o_sb")
                            nc.scalar.copy(o_sb, wo_ps)
                            nc.sync.dma_start(
                                out=outp[s0:s0 + 128, :], in_=o_sb)
    nc.compile()
    return nc


_NC_CACHE = {}
LAST_EXEC_NS = None
LAST_RESULT = None


def _get_nc():
    if "nc" not in _NC_CACHE:
        _NC_CACHE["nc"] = build_nc()
    return _NC_CACHE["nc"]


def make_in_maps(inputs):
    hs = np.asarray(inputs["hidden_states"], np.float32)
    au = np.asarray(inputs["au_embedding"], np.float32)
    Wq = np.asarray(inputs["Wq"], np.float32)
    Wk = np.asarray(inputs["Wk"], np.float32)
    Wv = np.asarray(inputs["Wv"], np.float32)
    Wak = np.asarray(inputs["Wak"], np.float32)
    Wav = np.asarray(inputs["Wav"], np.float32)
    null_token = np.asarray(inputs["null_token"], np.float32).reshape(1, C)
    gamma = np.asarray(inputs["gamma"], np.float32)
    Wo = np.asarray(inputs["Wo"], np.float32)

    Wq_s = Wq * SCALE
    Wav_g = Wav * gamma[None, :]

    def b16(x):
        return np.ascontiguousarray(x.astype(BF16_NP))

    in_maps = []
    for c in range(8):
        b, hg = divmod(c, 2)
        sl = slice(hg * CH, (hg + 1) * CH)
        ext = np.concatenate(
            [au[b], null_token, np.zeros((1, C), np.float32)], axis=0
        )  # [14, C]; row 13 is even-size padding
        extz = ext.copy()
        extz[NAU - 1] = 0.0
        in_maps.append({
            "hsT": b16(hs[b].T),
            "wq": b16(Wq_s[:, sl]),
            "wk": b16(Wk[:, sl]),
            "wv": b16(Wv[:, sl]),
            "wak": b16(Wak[:, sl]),
            "wav": b16(Wav_g[:, sl]),
            "wo": b16(Wo[sl, :]),
            "extT": b16(ext.T),
            "extzT": b16(extz.T),
        })
    return in_maps


def kernel(**inputs):
    global LAST_EXEC_NS, LAST_RESULT
    hs = np.asarray(inputs["hidden_states"], np.float32)
    bo = np.asarray(inputs["bo"], np.float32)
    in_maps = make_in_maps(inputs)
    nc = _get_nc()
    trace = os.environ.get("KERNEL_TRACE", "0") == "1"
    res = run_bass_kernel_spmd(nc, in_maps, list(range(8)), trace=trace)
    LAST_EXEC_NS = res.exec_time_ns
    LAST_RESULT = res
    out = np.empty((B, S, C), np.float32)
    for b in range(B):
        out[b] = res.results[2 * b]["outp"] + res.results[2 * b + 1]["outp"]
        out[b] += bo[None, :]
        out[b] += hs[b]
    return out


# revision 15
# speedup vs baseline: 1.4240x; 1.0440x over previous
"""Trainium2 Bass kernel for nn_AUAttnProcessor (AU-token attention processor).

Sharding: 8 cores = (batch b, head-group hg). Core c handles batch c//2 and
heads [4*(c%2), 4*(c%2)+4) (Ch=320 of C=640 channels).  Wq/Wk/Wv/Wak/Wav are
column-sharded, Wo row-sharded; each core emits a partial [S, C] output and the
host reduces the two partials per batch and adds bias + residual.

Per-core pipeline (transposed flash-attention orientation, bf16 operands,
inputs pre-cast to bf16 on the host):
  qT/kT = (W.T @ hsT)            [80, S] per head, evacuated bf16
  vaug  = hs @ Wv  + ones col 80 [128, sc, h, 82] bf16
  scoresT[kc] = kT_chunk.T @ qT  -> PSUM f32 [128, 1024]
  expT = Exp(scoresT) (ACT)      -> SBUF bf16
  outT += vaug_chunk.T @ expT    [82, 1024] PSUM; row 80 = softmax denominator
  raw_m = cast(outT) bf16        [82, NH, S] (row 80 = denominators)
  denominators: cast-DMA rows -> dsum[4,S] f32 -> reciprocal_approx_fast
                -> cast-DMA -> rrows [1, 2, NH, S] bf16 (partition 0)
  bc = partition_broadcast(rrows row)  (GpSimd, SBUF bf16)
  osb = raw * bc  (DVE bf16 2x mode)   per head and branch
  Wo: accumulate 16 matmuls (main+au, 4 heads) per 128-query chunk -> [128, 640]
"""

import os
import sys

import numpy as np

for _p in ("/opt/trn_rl_repo",):
    if os.path.isdir(_p) and _p not in sys.path:
        sys.path.insert(0, _p)

import ml_dtypes

import concourse.bass as bass
import concourse.tile as tile
from concourse import bacc, mybir
from concourse.bass_utils import run_bass_kernel_spmd

# Problem dims
B, S, C, H, D = 4, 2048, 640, 8, 80
NH = 4            # heads per core
CH = NH * D       # 320 channels per core
KC = C // 128     # 5 contraction chunks
SC = S // 128     # 16 sequence chunks
NAU = 13          # 12 AU tokens + 1 null token
NAUP = 14         # padded to even
VA = 82           # vaug rows: 80 v + ones row (80) + zero pad (81)
QB = 1024         # q-block width
NQB = S // QB
SCALE = float(D) ** -0.5

F32 = mybir.dt.float32
BF16 = mybir.dt.bfloat16
EXP = mybir.ActivationFunctionType.Exp
BF16_NP = ml_dtypes.bfloat16


def _phase_p(nc, tc, pers_tiles, raw_au, dram):
    """Projections + AU attention. Closes its pools on exit (frees hsT_sb)."""
    qT, kT, vaug, wo_sb, aukT, auvaug = pers_tiles
    hsT, wq, wk, wv, wak, wav, wo, extT, extzT = dram
    with tc.tile_pool(name="projp", bufs=1) as projp, \
         tc.tile_pool(name="wts", bufs=2) as wpool, \
         tc.tile_pool(name="ppsum", bufs=4, space="PSUM") as pps:
        ext_sb = projp.tile([128, KC, NAUP], BF16, name="ext_sb")
        nc.sync.dma_start(
            out=ext_sb, in_=extT[:].rearrange("(c p) n -> p c n", p=128))
        extz_sb = projp.tile([128, KC, NAUP], BF16, name="extz_sb")
        nc.sync.dma_start(
            out=extz_sb, in_=extzT[:].rearrange("(c p) n -> p c n", p=128))
        hsT_sb = projp.tile([128, KC, S], BF16, name="hsT_sb")
        nc.sync.dma_start(
            out=hsT_sb, in_=hsT[:].rearrange("(c p) s -> p c s", p=128))
        nc.sync.dma_start(
            out=wo_sb, in_=wo[:].rearrange("(h d) n -> d h n", d=D))

        # au_k projection first: tiny matmuls warm the PE while the hsT DMA
        # is in flight
        w_sb = wpool.tile([128, KC, CH], BF16, tag="w", name="wak_sb")
        nc.sync.dma_start(
            out=w_sb, in_=wak[:].rearrange("(c p) n -> p c n", p=128))
        for h in range(NH):
            ps = pps.tile([D, NAUP], F32, tag="pp", name="ps_auk")
            for c in range(KC):
                nc.tensor.matmul(
                    ps,
                    w_sb[:, c, h * D:(h + 1) * D],
                    ext_sb[:, c, :],
                    start=(c == 0), stop=(c == KC - 1),
                )
            nc.vector.tensor_copy(aukT[:, h, :], ps)

        # au_v projection (natural [14, 320], gamma pre-folded on host)
        w_sb = wpool.tile([128, KC, CH], BF16, tag="w", name="wav_sb")
        nc.sync.dma_start(
            out=w_sb, in_=wav[:].rearrange("(c p) n -> p c n", p=128))
        ps = pps.tile([NAUP, CH], F32, tag="pp", name="ps_auv")
        for c in range(KC):
            nc.tensor.matmul(
                ps,
                extz_sb[:, c, :],
                w_sb[:, c, :],
                start=(c == 0), stop=(c == KC - 1),
            )
        nc.vector.tensor_copy(
            auvaug[:, :, 0:80], ps.rearrange("p (h d) -> p h d", d=D))

        # q and k projections, per head (transposed output, bf16)
        for wdram, dstT, evac in ((wq, qT, "act"), (wk, kT, "dve")):
            w_sb = wpool.tile([128, KC, CH], BF16, tag="w", name="w_sb")
            nc.sync.dma_start(
                out=w_sb, in_=wdram[:].rearrange("(c p) n -> p c n", p=128))
            for h in range(NH):
                for nb in range(S // 512):
                    ps = pps.tile([D, 512], F32, tag="pp", name="ps_qk")
                    for c in range(KC):
                        nc.tensor.matmul(
                            ps,
                            w_sb[:, c, h * D:(h + 1) * D],
                            hsT_sb[:, c, nb * 512:(nb + 1) * 512],
                            start=(c == 0), stop=(c == KC - 1),
                        )
                    if evac == "act":
                        nc.scalar.copy(dstT[:, h, nb * 512:(nb + 1) * 512], ps)
                    else:
                        nc.vector.tensor_copy(
                            dstT[:, h, nb * 512:(nb + 1) * 512], ps)

        # v projection (natural layout, strided into vaug)
        w_sb = wpool.tile([128, KC, CH], BF16, tag="w", name="wv_sb")
        nc.sync.dma_start(
            out=w_sb, in_=wv[:].rearrange("(c p) n -> p c n", p=128))
        for sc in range(SC):
            ps = pps.tile([128, CH], F32, tag="pp", name="ps_v")
            for c in range(KC):
                nc.tensor.matmul(
                    ps,
                    hsT_sb[:, c, sc * 128:(sc + 1) * 128],
                    w_sb[:, c, :],
                    start=(c == 0), stop=(c == KC - 1),
                )
            nc.vector.tensor_copy(
                vaug[:, sc, :, 0:80], ps.rearrange("p (h d) -> p h d", d=D))


def build_nc():
    nc = bacc.Bacc()
    hsT = nc.dram_tensor("hsT", [C, S], BF16, kind="ExternalInput")
    wq = nc.dram_tensor("wq", [C, CH], BF16, kind="ExternalInput")
    wk = nc.dram_tensor("wk", [C, CH], BF16, kind="ExternalInput")
    wv = nc.dram_tensor("wv", [C, CH], BF16, kind="ExternalInput")
    wak = nc.dram_tensor("wak", [C, CH], BF16, kind="ExternalInput")
    wav = nc.dram_tensor("wav", [C, CH], BF16, kind="ExternalInput")
    wo = nc.dram_tensor("wo", [CH, C], BF16, kind="ExternalInput")
    extT = nc.dram_tensor("extT", [C, NAUP], BF16, kind="ExternalInput")
    extzT = nc.dram_tensor("extzT", [C, NAUP], BF16, kind="ExternalInput")
    outp = nc.dram_tensor("outp", [S, C], F32, kind="ExternalOutput")
    dram = (hsT, wq, wk, wv, wak, wav, wo, extT, extzT)

    with tile.TileContext(nc) as tc, \
         nc.allow_low_precision(reason="bf16 attention pipeline; approx recip"):
        with tc.tile_pool(name="pers", bufs=1) as pers:
            qT = pers.tile([D, NH, S], BF16, name="qT")
            kT = pers.tile([D, NH, S], BF16, name="kT")
            vaug = pers.tile([128, SC, NH, VA], BF16, name="vaug")
            wo_sb = pers.tile([D, NH, C], BF16, name="wo_sb")
            aukT = pers.tile([D, NH, NAUP], BF16, name="aukT")
            auvaug = pers.tile([NAUP, NH, VA], BF16, name="auvaug")

            nc.vector.memset(vaug[:, :, :, 80:81], 1.0)
            nc.vector.memset(vaug[:, :, :, 81:82], 0.0)
            # auvaug: col 80 = 1 on rows 0:13 only (pad row 13 excluded from
            # the denominator), col 81 = 0
            nc.vector.memset(auvaug[:, :, 80:82], 0.0)
            nc.vector.memset(auvaug[0:NAU, :, 80:81], 1.0)

            with tc.tile_pool(name="pmid", bufs=1) as pmid:
                raw_au = pmid.tile([VA, NH, S], BF16, name="raw_au")

                _phase_p(nc, tc, (qT, kT, vaug, wo_sb, aukT, auvaug),
                         raw_au, dram)

                # Persistent tiles for phases B/C (allocated after the
                # projection pool closed, reusing hsT_sb's space)
                with tc.tile_pool(name="pers2", bufs=1) as pers2, \
                     tc.tile_pool(name="bcp", bufs=3) as bcp:
                    raw_m = pers2.tile([VA, NH, S], BF16, name="raw_m")
                    osb_m = pers2.tile([D, NH, S], BF16, name="osb_m")
                    osb_a = pers2.tile([D, NH, S], BF16, name="osb_a")
                    dsum_m = pers2.tile([4, S], F32, name="dsum_m")
                    dsum_a = pers2.tile([4, S], F32, name="dsum_a")
                    dsum_rm = pers2.tile([4, S], F32, name="dsum_rm")
                    dsum_ra = pers2.tile([4, S], F32, name="dsum_ra")
                    rrows = pers2.tile([1, 2, NH, S], BF16, name="rrows")

                    # ---------------- Phase B: main attention ----------------
                    with tc.tile_pool(name="spool", bufs=2, space="PSUM") as spool, \
                         tc.tile_pool(name="opool", bufs=2, space="PSUM") as opool, \
                         tc.tile_pool(name="expp", bufs=3) as expp, \
                         tc.tile_pool(name="outp_sb", bufs=3) as outsb_pool:
                        # AU branch attention first, inside the same pools
                        # (no PSUM pool transition before the main loop)
                        for h in range(NH):
                            for hf in range(2):
                                f0 = hf * 1024
                                aus = spool.tile(
                                    [128, QB], F32, tag="sc", name="aus")
                                for nb in range(2):
                                    nc.tensor.matmul(
                                        aus[0:NAUP, nb * 512:(nb + 1) * 512],
                                        aukT[:, h, :],
                                        qT[:, h, f0 + nb * 512:f0 + (nb + 1) * 512],
                                        start=True, stop=True,
                                    )
                                au_e = expp.tile(
                                    [128, QB], BF16, tag="ex", name="au_e")
                                nc.scalar.activation(
                                    out=au_e[0:NAUP, :], in_=aus[0:NAUP, :],
                                    func=EXP)
                                auo = opool.tile(
                                    [VA, QB], F32, tag="ot", name="auo")
                                for nb in range(2):
                                    nc.tensor.matmul(
                                        auo[:, nb * 512:(nb + 1) * 512],
                                        auvaug[:, h, :],
                                        au_e[0:NAUP, nb * 512:(nb + 1) * 512],
                                        start=True, stop=True,
                                    )
                                nc.vector.tensor_copy(
                                    raw_au[:, h, f0:f0 + 1024], auo)

                        # au denominator pipeline + merge (overlaps early
                        # main attention)
                        nc.gpsimd.dma_start(out=dsum_a, in_=raw_au[80:81, :, :])
                        nc.vector.reciprocal_approx_fast(
                            out=dsum_ra, in_=dsum_a)
                        nc.gpsimd.dma_start(
                            out=rrows[0:1, 1, :, :], in_=dsum_ra)
                        for h in range(NH):
                            bc = bcp.tile([D, S], BF16, tag="bca", name="bc_a")
                            nc.gpsimd.partition_broadcast(
                                bc, rrows[0:1, 1, h, :])
                            nc.vector.tensor_mul(
                                osb_a[:, h, :], raw_au[0:80, h, :], bc)

                        def emit_wo(sj, outsb_pool, opool):
                            # Wo projection for query chunk sj; wo_ps shares
                            # the "ot" PSUM ring with outT (same pool/banks)
                            s0 = sj * 128
                            wo_ps = opool.tile(
                                [128, C], F32, tag="ot", name="wo_ps")
                            for n0, n1 in ((0, 512), (512, 640)):
                                k = 0
                                for osb in (osb_m, osb_a):
                                    for h in range(NH):
                                        nc.tensor.matmul(
                                            wo_ps[:, n0:n1],
                                            osb[:, h, s0:s0 + 128],
                                            wo_sb[:, h, n0:n1],
                                            start=(k == 0), stop=(k == 7),
                                        )
                                        k += 1
                            o_sb = outsb_pool.tile(
                                [128, C], F32, tag="ob", name="o_sb")
                            nc.vector.tensor_copy(o_sb, wo_ps)
                            nc.sync.dma_start(
                                out=outp[s0:s0 + 128, :], in_=o_sb)

                        for qb in range(NQB):
                            q0 = qb * QB
                            for h in range(NH):
                                outT = opool.tile(
                                    [VA, QB], F32, tag="ot", name="outT")
                                for kc in range(SC):
                                    sco = spool.tile(
                                        [128, QB], F32, tag="sc", name="sco")
                                    for nn in range(QB // 512):
                                        nc.tensor.matmul(
                                            sco[:, nn * 512:(nn + 1) * 512],
                                            kT[:, h, kc * 128:(kc + 1) * 128],
                                            qT[:, h,
                                               q0 + nn * 512:q0 + (nn + 1) * 512],
                                            start=True, stop=True,
                                        )
                                    ex = expp.tile(
                                        [128, QB], BF16, tag="ex", name="ex")
                                    nc.scalar.activation(
                                        out=ex, in_=sco, func=EXP)
                                    for nn in range(QB // 512):
                                        nc.tensor.matmul(
                                            outT[:, nn * 512:(nn + 1) * 512],
                                            vaug[:, kc, h, :],
                                            ex[:, nn * 512:(nn + 1) * 512],
                                            start=(kc == 0), stop=(kc == SC - 1),
                                        )
                                nc.vector.tensor_copy(
                                    raw_m[:, h, q0:q0 + QB], outT)
                                if qb == 1:
                                    # interleave qb0's Wo into qb1's attention
                                    # (fills the ACT-paced PE slack, keeps
                                    # the PE warm through the output phase)
                                    emit_wo(2 * h, outsb_pool, opool)
                                    emit_wo(2 * h + 1, outsb_pool, opool)

                            # denominator pipeline + merge for this q-block
                            # (overlaps the next q-block's attention)
                            nc.gpsimd.dma_start(
                                out=dsum_m[:, q0:q0 + QB],
                                in_=raw_m[80:81, :, q0:q0 + QB],
                            )
                            nc.vector.reciprocal_approx_fast(
                                out=dsum_rm[:, q0:q0 + QB],
                                in_=dsum_m[:, q0:q0 + QB])
                            nc.gpsimd.dma_start(
                                out=rrows[0:1, 0, :, q0:q0 + QB],
                                in_=dsum_rm[:, q0:q0 + QB],
                            )
                            for h in range(NH):
                                bc = bcp.tile(
                                    [D, QB], BF16, tag="bcm", name="bc_m")
                                nc.gpsimd.partition_broadcast(
                                    bc, rrows[0:1, 0, h, q0:q0 + QB])
                                nc.vector.tensor_mul(
                                    osb_m[:, h, q0:q0 + QB],
                                    raw_m[0:80, h, q0:q0 + QB],
                                    bc,
                                )

                        # tail: qb1's Wo chunks
                        for sj in range(8, S // 128):
                            emit_wo(sj, outsb_pool, opool)
    nc.compile()
    return nc


_NC_CACHE = {}
LAST_EXEC_NS = None
LAST_RESULT = None


def _get_nc():
    if "nc" not in _NC_CACHE:
        _NC_CACHE["nc"] = build_nc()
    return _NC_CACHE["nc"]


def make_in_maps(inputs):
    hs = np.asarray(inputs["hidden_states"], np.float32)
    au = np.asarray(inputs["au_embedding"], np.float32)
    Wq = np.asarray(inputs["Wq"], np.float32)
    Wk = np.asarray(inputs["Wk"], np.float32)
    Wv = np.asarray(inputs["Wv"], np.float32)
    Wak = np.asarray(inputs["Wak"], np.float32)
    Wav = np.asarray(inputs["Wav"], np.float32)
    null_token = np.asarray(inputs["null_token"], np.float32).reshape(1, C)
    gamma = np.asarray(inputs["gamma"], np.float32)
    Wo = np.asarray(inputs["Wo"], np.float32)

    Wq_s = Wq * SCALE
    Wav_g = Wav * gamma[None, :]

    def b16(x):
        return np.ascontiguousarray(x.astype(BF16_NP))

    in_maps = []
    for c in range(8):
        b, hg = divmod(c, 2)
        sl = slice(hg * CH, (hg + 1) * CH)
        ext = np.concatenate(
            [au[b], null_token, np.zeros((1, C), np.float32)], axis=0
        )  # [14, C]; row 13 is even-size padding
        extz = ext.copy()
        extz[NAU - 1] = 0.0
        in_maps.append({
            "hsT": b16(hs[b].T),
            "wq": b16(Wq_s[:, sl]),
            "wk": b16(Wk[:, sl]),
            "wv": b16(Wv[:, sl]),
            "wak": b16(Wak[:, sl]),
            "wav": b16(Wav_g[:, sl]),
            "wo": b16(Wo[sl, :]),
            "extT": b16(ext.T),
            "extzT": b16(extz.T),
        })
    return in_maps


def kernel(**inputs):
    global LAST_EXEC_NS, LAST_RESULT
    hs = np.asarray(inputs["hidden_states"], np.float32)
    bo = np.asarray(inputs["bo"], np.float32)
    in_maps = make_in_maps(inputs)
    nc = _get_nc()
    trace = os.environ.get("KERNEL_TRACE", "0") == "1"
    res = run_bass_kernel_spmd(nc, in_maps, list(range(8)), trace=trace)
    LAST_EXEC_NS = res.exec_time_ns
    LAST_RESULT = res
    out = np.empty((B, S, C), np.float32)
    for b in range(B):
        out[b] = res.results[2 * b]["outp"] + res.results[2 * b + 1]["outp"]
        out[b] += bo[None, :]
        out[b] += hs[b]
    return out
